# revision 10
# baseline (speedup 1.0000x reference)
"""DBN-Sigma whitening (group-wise decorrelated batch norm) on 8 trn2 cores.

Strategy (data-parallel over batch N, hint-conformant):
  Pass A (device): each core takes 8 of 64 images; computes per-channel
    sums S1 and the two diagonal 128x128 blocks of the raw second moment
    S2 = sum_m x x^T (only those cover the 16 per-group 16x16 sigmas).
    x is cast once to bf16 (ACT engine, fused row-sum via accum_out);
    m-chunks are transposed to [m, c] layout either on the PE (bf16
    transpose -> bf16 PSUM -> DVE copy) or via the DMA xbar
    (dma_start_transpose, 3D out) -- split tuned so PE and DMA balance;
    cov accumulates over all chunks in PSUM via bf16 matmuls.
  Host: reduce partials over cores (f64), sigma_g = S2_g/m - mean mean^T
    + eps I per 16-channel group, eigh -> wm_g = sigma_g^{-1/2}; fold
    mean subtraction and weight/bias into a per-channel affine.
  Pass B (device, pure f32): out = scale_c * (wm @ x)_c + shift_c,
    streamed with 2-image DMAs; affine applied on the scalar engine
    during the PSUM->SBUF move.

Layout: X [64, 256, 56*56] f32; channels on SBUF partitions (2 halves
of 128), free dim = pixel index m. Per-core m = 8*3136; image pairs
give 6272 = 49*128 exactly (no remainder chunks).
"""

import numpy as np
import ml_dtypes
import concourse.bass as bass
import concourse.bacc as bacc
import concourse.mybir as mybir
import concourse.tile as tile
from concourse.bass_utils import run_bass_kernel_spmd

N_CORES = 8
N, C, H, W = 64, 256, 56, 56
HW = H * W                     # 3136
NL = N // N_CORES              # 8 images per core
G, CG = 16, 16
EPS = 1e-3
M_TOT = N * HW
FP = mybir.dt.float32
BF = mybir.dt.bfloat16

NP_ = NL // 2                  # 4 image pairs per core
FPAIR = 2 * HW                 # 6272 free elems per (pair, half)
NCH = FPAIR // 128             # 49 m-chunks per (pair, half)

# Which of the 8 (pair, half) units route their transposes through the
# DMA xbar instead of the PE (balances PE vs DMA time in pass A).
DMA_T_UNITS = {2, 5}


def _build_pass_a():
    nc = bacc.Bacc("TRN2", target_bir_lowering=False, debug=False,
                   num_devices=N_CORES)
    X_d = nc.dram_tensor("X", [NL, C, HW], BF, kind="ExternalInput")
    eye_d = nc.dram_tensor("eye", [128, 128], BF, kind="ExternalInput")
    S1_d = nc.dram_tensor("S1", [128, 2], FP, kind="ExternalOutput")
    S2_d = nc.dram_tensor("S2", [2, 128, 128], FP, kind="ExternalOutput")
    X = X_d.ap()

    with tile.TileContext(nc) as tc:
        with (
            tc.tile_pool(name="const", bufs=1) as constp,
            tc.tile_pool(name="xbf", bufs=3) as xbp,
            tc.tile_pool(name="xbt", bufs=2) as xbtp,
            tc.tile_pool(name="xtq", bufs=4) as xtqp,
            tc.tile_pool(name="red", bufs=2) as redp,
            tc.tile_pool(name="acc", bufs=1) as accp,
            tc.tile_pool(name="ptp", bufs=3, space="PSUM") as ptp,
            tc.tile_pool(name="cov", bufs=1, space="PSUM") as covp,
        ):
            eye = constp.tile([128, 128], BF)
            nc.sync.dma_start(eye[:], eye_d.ap())
            s1 = accp.tile([128, 2], FP)
            nc.vector.memset(s1[:], 0.0)
            cov = [covp.tile([128, 128], FP, tag=f"cov{h}", name=f"cov{h}")
                   for h in (0, 1)]
            started = [False, False]

            for p in range(NP_):
                for h in (0, 1):
                    u = p * 2 + h
                    xb = xbp.tile([128, FPAIR], BF, tag="xb")
                    for i in (0, 1):
                        nc.sync.dma_start(
                            xb[:, HW * i:HW * (i + 1)],
                            X[2 * p + i, 128 * h:128 * (h + 1), :])
                    r = redp.tile([128, 1], FP, tag="r")
                    scr = redp.tile([128, FPAIR], BF, tag="scr", bufs=1)
                    nc.scalar.activation(scr[:], xb[:],
                                         mybir.ActivationFunctionType.Copy,
                                         accum_out=r[:])
                    nc.vector.tensor_add(s1[:, h:h + 1], s1[:, h:h + 1], r[:])

                    last_u = (p == NP_ - 1)
                    if u in DMA_T_UNITS:
                        xbT = xbtp.tile([128, NCH, 128], BF, tag="xbT")
                        nc.sync.dma_start_transpose(xbT[:], xb[:])
                        for j in range(NCH):
                            sl = xbT[:, j, :]
                            nc.tensor.matmul(
                                cov[h][:], sl, sl,
                                start=not started[h],
                                stop=last_u and j == NCH - 1,
                                skip_group_check=True)
                            started[h] = True
                    else:
                        for q in range(13):        # 49 = 12*4 + 1 chunks
                            nch = 4 if q < 12 else 1
                            pt = ptp.tile([128, nch * 128], BF, tag="pt")
                            for jj in range(nch):
                                m0 = 128 * (4 * q + jj)
                                nc.tensor.transpose(
                                    pt[:, 128 * jj:128 * (jj + 1)],
                                    xb[:, m0:m0 + 128], eye[:])
                            xtq = xtqp.tile([128, nch * 128], BF, tag="xtq")
                            nc.vector.tensor_copy(xtq[:], pt[:])
                            for jj in range(nch):
                                sl = xtq[:, 128 * jj:128 * (jj + 1)]
                                nc.tensor.matmul(
                                    cov[h][:], sl, sl,
                                    start=not started[h],
                                    stop=(last_u and q == 12 and jj == nch - 1),
                                    skip_group_check=True)
                                started[h] = True

            s2sb = accp.tile([128, 256], FP)
            for h in (0, 1):
                nc.vector.tensor_copy(s2sb[:, 128 * h:128 * (h + 1)], cov[h][:])
                nc.sync.dma_start(S2_d.ap()[h], s2sb[:, 128 * h:128 * (h + 1)])
            nc.sync.dma_start(S1_d.ap(), s1[:])

    nc.compile()
    return nc


def _build_pass_b():
    nc = bacc.Bacc("TRN2", target_bir_lowering=False, debug=False,
                   num_devices=N_CORES)
    X_d = nc.dram_tensor("X", [NL, C, HW], FP, kind="ExternalInput")
    wm_d = nc.dram_tensor("wm", [128, 256], FP, kind="ExternalInput")
    sc_d = nc.dram_tensor("sc", [128, 2], FP, kind="ExternalInput")
    sh_d = nc.dram_tensor("sh", [128, 2], FP, kind="ExternalInput")
    Xn_d = nc.dram_tensor("Xn", [NL, C, HW], FP, kind="ExternalOutput")
    X = X_d.ap()
    Xn = Xn_d.ap()

    KT = 448                   # matmul free-dim tile (14 * 448 = 6272)
    NK = FPAIR // KT

    with tile.TileContext(nc) as tc:
        with (
            tc.tile_pool(name="const", bufs=1) as constp,
            tc.tile_pool(name="xin", bufs=3) as xp,
            tc.tile_pool(name="xout", bufs=3) as op,
            tc.tile_pool(name="ps", bufs=4, space="PSUM") as psp,
        ):
            wm = constp.tile([128, 256], FP)
            nc.sync.dma_start(wm[:], wm_d.ap())
            sc = constp.tile([128, 2], FP)
            nc.sync.dma_start(sc[:], sc_d.ap())
            sh = constp.tile([128, 2], FP)
            nc.sync.dma_start(sh[:], sh_d.ap())

            for h in (0, 1):
                for p in range(NP_):
                    xf = xp.tile([128, FPAIR], FP, tag="x")
                    for i in (0, 1):
                        nc.sync.dma_start(
                            xf[:, HW * i:HW * (i + 1)],
                            X[2 * p + i, 128 * h:128 * (h + 1), :])
                    ot = op.tile([128, FPAIR], FP, tag="o")
                    for k in range(NK):
                        ps = psp.tile([128, KT], FP, tag="ps")
                        nc.tensor.matmul(
                            ps[:], wm[:, 128 * h:128 * (h + 1)],
                            xf[:, KT * k:KT * (k + 1)])
                        nc.scalar.activation(
                            ot[:, KT * k:KT * (k + 1)], ps[:],
                            mybir.ActivationFunctionType.Identity,
                            bias=sh[:, h:h + 1], scale=sc[:, h:h + 1])
                    for i in (0, 1):
                        nc.sync.dma_start(
                            Xn[2 * p + i, 128 * h:128 * (h + 1), :],
                            ot[:, HW * i:HW * (i + 1)])

    nc.compile()
    return nc


_PROGS = {}


def _programs():
    if "a" not in _PROGS:
        _PROGS["a"] = _build_pass_a()
        _PROGS["b"] = _build_pass_b()
    return _PROGS["a"], _PROGS["b"]


def kernel(X, weight, bias, _return_results=False):
    X = np.asarray(X, dtype=np.float32)
    weight = np.asarray(weight, dtype=np.float32).reshape(C)
    bias = np.asarray(bias, dtype=np.float32).reshape(C)
    nc_a, nc_b = _programs()

    Xr = X.reshape(N, C, HW)
    shards = [Xr[NL * i:NL * (i + 1)] for i in range(N_CORES)]
    shards_bf = [s.astype(ml_dtypes.bfloat16) for s in shards]
    eye = np.eye(128, dtype=ml_dtypes.bfloat16)
    core_ids = list(range(N_CORES))

    res_a = run_bass_kernel_spmd(
        nc_a, [{"X": s, "eye": eye} for s in shards_bf], core_ids)

    # host reduction of the tiny per-core stats (f64 for cleanliness)
    s1 = np.zeros((128, 2), np.float64)
    s2 = np.zeros((2, 128, 128), np.float64)
    for r in res_a.results:
        s1 += r["S1"].astype(np.float64)
        s2 += r["S2"].astype(np.float64)

    mean = np.concatenate([s1[:, 0], s1[:, 1]]) / M_TOT          # [256]
    wm_bd = np.zeros((2, 128, 128), np.float64)
    for g in range(G):
        h, o = divmod(g, 128 // CG)
        o *= CG
        mg = mean[CG * g:CG * (g + 1)]
        sg = (s2[h][o:o + CG, o:o + CG] / M_TOT - np.outer(mg, mg)
              + EPS * np.eye(CG))
        lam, u = np.linalg.eigh(sg)
        wm_bd[h][o:o + CG, o:o + CG] = (u / np.sqrt(lam)) @ u.T

    wm_full = np.zeros((C, C), np.float64)
    wm_full[:128, :128] = wm_bd[0]
    wm_full[128:, 128:] = wm_bd[1]
    v = wm_full @ mean                                           # [256]
    scale = weight.astype(np.float64)
    shift = bias.astype(np.float64) - scale * v

    wm_in = np.concatenate([wm_bd[0], wm_bd[1]], axis=1).astype(np.float32)
    sc_in = np.stack([scale[:128], scale[128:]], axis=1).astype(np.float32)
    sh_in = np.stack([shift[:128], shift[128:]], axis=1).astype(np.float32)

    res_b = run_bass_kernel_spmd(
        nc_b,
        [{"X": s, "wm": wm_in, "sc": sc_in, "sh": sh_in} for s in shards],
        core_ids)

    out = np.concatenate([r["Xn"] for r in res_b.results], axis=0)
    out = out.reshape(N, C, H, W).astype(np.float32)
    if _return_results:
        return out, (res_a, res_b)
    return out


# revision 15
# speedup vs baseline: 1.3517x; 1.3517x over previous
"""DBN-Sigma whitening (group-wise decorrelated batch norm) on 8 trn2 cores.

Strategy (data-parallel over batch N, hint-conformant):
  Pass A (device): each core takes 8 of 64 images; computes per-channel
    sums S1 and the two diagonal 128x128 blocks of the raw second moment
    S2 = sum_m x x^T (only those cover the 16 per-group 16x16 sigmas).
    x is cast once to bf16 (ACT engine, fused row-sum via accum_out);
    m-chunks are transposed to [m, c] layout either on the PE (bf16
    transpose -> bf16 PSUM -> DVE copy) or via the DMA xbar
    (dma_start_transpose, 3D out) -- split tuned so PE and DMA balance;
    cov accumulates over all chunks in PSUM via bf16 matmuls.
  Host: reduce partials over cores (f64), sigma_g = S2_g/m - mean mean^T
    + eps I per 16-channel group, eigh -> wm_g = sigma_g^{-1/2}; fold
    mean subtraction and weight/bias into a per-channel affine.
  Pass B (device, pure f32): out = scale_c * (wm @ x)_c + shift_c,
    streamed with 2-image DMAs; affine applied on the scalar engine
    during the PSUM->SBUF move.

Layout: X [64, 256, 56*56] f32; channels on SBUF partitions (2 halves
of 128), free dim = pixel index m. Per-core m = 8*3136; image pairs
give 6272 = 49*128 exactly (no remainder chunks).
"""

import numpy as np
import ml_dtypes
import concourse.bass as bass
import concourse.bacc as bacc
import concourse.mybir as mybir
import concourse.tile as tile
from concourse.bass_utils import run_bass_kernel_spmd

N_CORES = 8
N, C, H, W = 64, 256, 56, 56
HW = H * W                     # 3136
NL = N // N_CORES              # 8 images per core
G, CG = 16, 16
EPS = 1e-3
M_TOT = N * HW
FP = mybir.dt.float32
BF = mybir.dt.bfloat16

NP_ = NL // 2                  # 4 image pairs per core
FPAIR = 2 * HW                 # 6272 free elems per (pair, half)
NCH = FPAIR // 128             # 49 m-chunks per (pair, half)

# Which of the 8 (pair, half) units route their transposes through the
# DMA xbar instead of the PE (balances PE vs DMA time in pass A).
DMA_T_UNITS = {2, 5}


def _build_pass_a():
    nc = bacc.Bacc("TRN2", target_bir_lowering=False, debug=False,
                   num_devices=N_CORES)
    X_d = nc.dram_tensor("X", [NL, C, HW], BF, kind="ExternalInput")
    eye_d = nc.dram_tensor("eye", [128, 128], BF, kind="ExternalInput")
    S1_d = nc.dram_tensor("S1", [128, 2], FP, kind="ExternalOutput")
    S2_d = nc.dram_tensor("S2", [2, 128, 128], FP, kind="ExternalOutput")
    X = X_d.ap()

    with tile.TileContext(nc) as tc:
        with (
            tc.tile_pool(name="const", bufs=1) as constp,
            tc.tile_pool(name="xbf", bufs=4) as xbp,
            tc.tile_pool(name="xbt", bufs=2) as xbtp,
            tc.tile_pool(name="xtq", bufs=6) as xtqp,
            tc.tile_pool(name="red", bufs=2) as redp,
            tc.tile_pool(name="acc", bufs=1) as accp,
            tc.tile_pool(name="ptp", bufs=4, space="PSUM") as ptp,
            tc.tile_pool(name="cov", bufs=1, space="PSUM") as covp,
        ):
            eye = constp.tile([128, 128], BF)
            nc.sync.dma_start(eye[:], eye_d.ap())
            s1 = accp.tile([128, 2], FP)
            nc.vector.memset(s1[:], 0.0)
            cov = [covp.tile([128, 128], FP, tag=f"cov{h}", name=f"cov{h}")
                   for h in (0, 1)]
            started = [False, False]

            for p in range(NP_):
                for h in (0, 1):
                    u = p * 2 + h
                    xb = xbp.tile([128, FPAIR], BF, tag="xb")
                    for i in (0, 1):
                        nc.sync.dma_start(
                            xb[:, HW * i:HW * (i + 1)],
                            X[2 * p + i, 128 * h:128 * (h + 1), :])
                    r = redp.tile([128, 1], FP, tag="r")
                    scr = redp.tile([128, FPAIR], BF, tag="scr", bufs=1)
                    nc.scalar.activation(scr[:], xb[:],
                                         mybir.ActivationFunctionType.Copy,
                                         accum_out=r[:])
                    nc.vector.tensor_add(s1[:, h:h + 1], s1[:, h:h + 1], r[:])

                    last_u = (p == NP_ - 1)
                    if u in DMA_T_UNITS:
                        xbT = xbtp.tile([128, NCH, 128], BF, tag="xbT")
                        nc.sync.dma_start_transpose(xbT[:], xb[:])
                        for j in range(NCH):
                            sl = xbT[:, j, :]
                            nc.tensor.matmul(
                                cov[h][:], sl, sl,
                                start=not started[h],
                                stop=last_u and j == NCH - 1,
                                skip_group_check=True)
                            started[h] = True
                    else:
                        for q in range(13):        # 49 = 12*4 + 1 chunks
                            nch = 4 if q < 12 else 1
                            pt = ptp.tile([128, nch * 128], BF, tag="pt")
                            for jj in range(nch):
                                m0 = 128 * (4 * q + jj)
                                nc.tensor.transpose(
                                    pt[:, 128 * jj:128 * (jj + 1)],
                                    xb[:, m0:m0 + 128], eye[:])
                            xtq = xtqp.tile([128, nch * 128], BF, tag="xtq")
                            nc.vector.tensor_copy(xtq[:], pt[:])
                            for jj in range(nch):
                                sl = xtq[:, 128 * jj:128 * (jj + 1)]
                                nc.tensor.matmul(
                                    cov[h][:], sl, sl,
                                    start=not started[h],
                                    stop=(last_u and q == 12 and jj == nch - 1),
                                    skip_group_check=True)
                                started[h] = True

            s2sb = accp.tile([128, 256], FP)
            for h in (0, 1):
                nc.vector.tensor_copy(s2sb[:, 128 * h:128 * (h + 1)], cov[h][:])
                nc.sync.dma_start(S2_d.ap()[h], s2sb[:, 128 * h:128 * (h + 1)])
            nc.sync.dma_start(S1_d.ap(), s1[:])

    nc.compile()
    return nc


def _build_pass_b():
    nc = bacc.Bacc("TRN2", target_bir_lowering=False, debug=False,
                   num_devices=N_CORES)
    X_d = nc.dram_tensor("X", [NL, C, HW], FP, kind="ExternalInput")
    wm_d = nc.dram_tensor("wm", [128, 256], FP, kind="ExternalInput")
    sc_d = nc.dram_tensor("sc", [128, 2], FP, kind="ExternalInput")
    sh_d = nc.dram_tensor("sh", [128, 2], FP, kind="ExternalInput")
    Xn_d = nc.dram_tensor("Xn", [NL, C, HW], FP, kind="ExternalOutput")
    X = X_d.ap()
    Xn = Xn_d.ap()

    KT = 448                   # matmul free-dim tile (14 * 448 = 6272)
    NK = FPAIR // KT

    with tile.TileContext(nc) as tc:
        with (
            tc.tile_pool(name="const", bufs=1) as constp,
            tc.tile_pool(name="xin", bufs=3) as xp,
            tc.tile_pool(name="xout", bufs=3) as op,
            tc.tile_pool(name="ps", bufs=6, space="PSUM") as psp,
        ):
            wm = constp.tile([128, 256], FP)
            nc.sync.dma_start(wm[:], wm_d.ap())
            sc = constp.tile([128, 2], FP)
            nc.sync.dma_start(sc[:], sc_d.ap())
            sh = constp.tile([128, 2], FP)
            nc.sync.dma_start(sh[:], sh_d.ap())

            HHW = HW // 2
            for h in (0, 1):
                for p in range(NP_):
                    xf = xp.tile([128, FPAIR], FP, tag="x")
                    cs = X[:, 128 * h:128 * (h + 1), :]
                    if h == 0 and p == 0:
                        # quarter-granularity on the first tile: shorter
                        # pipeline-fill head
                        for i in (0, 1):
                            for j in (0, 1):
                                nc.sync.dma_start(
                                    xf[:, HW * i + HHW * j:
                                       HW * i + HHW * (j + 1)],
                                    cs[2 * p + i, :, HHW * j:HHW * (j + 1)])
                    else:
                        for i in (0, 1):
                            nc.sync.dma_start(xf[:, HW * i:HW * (i + 1)],
                                              cs[2 * p + i, :, :])
                    ot = op.tile([128, FPAIR], FP, tag="o")
                    for k in range(NK):
                        ps = psp.tile([128, KT], FP, tag="ps")
                        nc.tensor.matmul(
                            ps[:], wm[:, 128 * h:128 * (h + 1)],
                            xf[:, KT * k:KT * (k + 1)])
                        nc.scalar.activation(
                            ot[:, KT * k:KT * (k + 1)], ps[:],
                            mybir.ActivationFunctionType.Identity,
                            bias=sh[:, h:h + 1], scale=sc[:, h:h + 1])
                    co = Xn[:, 128 * h:128 * (h + 1), :]
                    if h == 1 and p == NP_ - 1:
                        # quarter-granularity on the last store: shorter tail
                        for i in (0, 1):
                            for j in (0, 1):
                                nc.sync.dma_start(
                                    co[2 * p + i, :, HHW * j:HHW * (j + 1)],
                                    ot[:, HW * i + HHW * j:
                                       HW * i + HHW * (j + 1)])
                    else:
                        for i in (0, 1):
                            nc.sync.dma_start(co[2 * p + i, :, :],
                                              ot[:, HW * i:HW * (i + 1)])

    nc.compile()
    return nc


_PROGS = {}


def _programs():
    if "a" not in _PROGS:
        _PROGS["a"] = _build_pass_a()
        _PROGS["b"] = _build_pass_b()
    return _PROGS["a"], _PROGS["b"]


def kernel(X, weight, bias, _return_results=False):
    X = np.asarray(X, dtype=np.float32)
    weight = np.asarray(weight, dtype=np.float32).reshape(C)
    bias = np.asarray(bias, dtype=np.float32).reshape(C)
    nc_a, nc_b = _programs()

    Xr = X.reshape(N, C, HW)
    shards = [Xr[NL * i:NL * (i + 1)] for i in range(N_CORES)]
    shards_bf = [s.astype(ml_dtypes.bfloat16) for s in shards]
    eye = np.eye(128, dtype=ml_dtypes.bfloat16)
    core_ids = list(range(N_CORES))

    res_a = run_bass_kernel_spmd(
        nc_a, [{"X": s, "eye": eye} for s in shards_bf], core_ids)

    # host reduction of the tiny per-core stats (f64 for cleanliness)
    s1 = np.zeros((128, 2), np.float64)
    s2 = np.zeros((2, 128, 128), np.float64)
    for r in res_a.results:
        s1 += r["S1"].astype(np.float64)
        s2 += r["S2"].astype(np.float64)

    mean = np.concatenate([s1[:, 0], s1[:, 1]]) / M_TOT          # [256]
    wm_bd = np.zeros((2, 128, 128), np.float64)
    for g in range(G):
        h, o = divmod(g, 128 // CG)
        o *= CG
        mg = mean[CG * g:CG * (g + 1)]
        sg = (s2[h][o:o + CG, o:o + CG] / M_TOT - np.outer(mg, mg)
              + EPS * np.eye(CG))
        lam, u = np.linalg.eigh(sg)
        wm_bd[h][o:o + CG, o:o + CG] = (u / np.sqrt(lam)) @ u.T

    wm_full = np.zeros((C, C), np.float64)
    wm_full[:128, :128] = wm_bd[0]
    wm_full[128:, 128:] = wm_bd[1]
    v = wm_full @ mean                                           # [256]
    scale = weight.astype(np.float64)
    shift = bias.astype(np.float64) - scale * v

    wm_in = np.concatenate([wm_bd[0], wm_bd[1]], axis=1).astype(np.float32)
    sc_in = np.stack([scale[:128], scale[128:]], axis=1).astype(np.float32)
    sh_in = np.stack([shift[:128], shift[128:]], axis=1).astype(np.float32)

    res_b = run_bass_kernel_spmd(
        nc_b,
        [{"X": s, "wm": wm_in, "sc": sc_in, "sh": sh_in} for s in shards],
        core_ids)

    out = np.concatenate([r["Xn"] for r in res_b.results], axis=0)
    out = out.reshape(N, C, H, W).astype(np.float32)
    if _return_results:
        return out, (res_a, res_b)
    return out


# revision 18
# speedup vs baseline: 1.3729x; 1.0156x over previous
"""DBN-Sigma whitening (group-wise decorrelated batch norm) on 8 trn2 cores.

Strategy (data-parallel over batch N, hint-conformant):
  Pass A (device): each core takes 8 of 64 images; computes per-channel
    sums S1 and the two diagonal 128x128 blocks of the raw second moment
    S2 = sum_m x x^T (only those cover the 16 per-group 16x16 sigmas).
    x is cast once to bf16 (ACT engine, fused row-sum via accum_out);
    m-chunks are transposed to [m, c] layout either on the PE (bf16
    transpose -> bf16 PSUM -> DVE copy) or via the DMA xbar
    (dma_start_transpose, 3D out) -- split tuned so PE and DMA balance;
    cov accumulates over all chunks in PSUM via bf16 matmuls.
  Host: reduce partials over cores (f64), sigma_g = S2_g/m - mean mean^T
    + eps I per 16-channel group, eigh -> wm_g = sigma_g^{-1/2}; fold
    mean subtraction and weight/bias into a per-channel affine.
  Pass B (device, pure f32): out = scale_c * (wm @ x)_c + shift_c,
    streamed with 2-image DMAs; affine applied on the scalar engine
    during the PSUM->SBUF move.

Layout: X [64, 256, 56*56] f32; channels on SBUF partitions (2 halves
of 128), free dim = pixel index m. Per-core m = 8*3136; image pairs
give 6272 = 49*128 exactly (no remainder chunks).
"""

import numpy as np
import ml_dtypes
import concourse.bass as bass
import concourse.bacc as bacc
import concourse.mybir as mybir
import concourse.tile as tile
from concourse.bass_utils import run_bass_kernel_spmd

N_CORES = 8
N, C, H, W = 64, 256, 56, 56
HW = H * W                     # 3136
NL = N // N_CORES              # 8 images per core
G, CG = 16, 16
EPS = 1e-3
M_TOT = N * HW
FP = mybir.dt.float32
BF = mybir.dt.bfloat16

NP_ = NL // 2                  # 4 image pairs per core
FPAIR = 2 * HW                 # 6272 free elems per (pair, half)
NCH = FPAIR // 128             # 49 m-chunks per (pair, half)

# Which of the 8 (pair, half) units route their transposes through the
# DMA xbar instead of the PE (balances PE vs DMA time in pass A).
DMA_T_UNITS = {2, 5}


def _build_pass_a():
    nc = bacc.Bacc("TRN2", target_bir_lowering=False, debug=False,
                   num_devices=N_CORES)
    X_d = nc.dram_tensor("X", [NL, C, HW], BF, kind="ExternalInput")
    eye_d = nc.dram_tensor("eye", [128, 128], BF, kind="ExternalInput")
    S1_d = nc.dram_tensor("S1", [128, 2], FP, kind="ExternalOutput")
    S2_d = nc.dram_tensor("S2", [2, 128, 128], FP, kind="ExternalOutput")
    X = X_d.ap()

    with tile.TileContext(nc) as tc:
        with (
            tc.tile_pool(name="const", bufs=1) as constp,
            tc.tile_pool(name="xbf", bufs=4) as xbp,
            tc.tile_pool(name="xbt", bufs=2) as xbtp,
            tc.tile_pool(name="xtq", bufs=6) as xtqp,
            tc.tile_pool(name="red", bufs=2) as redp,
            tc.tile_pool(name="acc", bufs=1) as accp,
            tc.tile_pool(name="ptp", bufs=4, space="PSUM") as ptp,
            tc.tile_pool(name="cov", bufs=1, space="PSUM") as covp,
        ):
            eye = constp.tile([128, 128], BF)
            nc.sync.dma_start(eye[:], eye_d.ap())
            s1 = accp.tile([128, 2], FP)
            nc.vector.memset(s1[:], 0.0)
            cov = [covp.tile([128, 128], FP, tag=f"cov{h}", name=f"cov{h}")
                   for h in (0, 1)]
            started = [False, False]

            for p in range(NP_):
                for h in (0, 1):
                    u = p * 2 + h
                    xb = xbp.tile([128, FPAIR], BF, tag="xb")
                    for i in (0, 1):
                        nc.sync.dma_start(
                            xb[:, HW * i:HW * (i + 1)],
                            X[2 * p + i, 128 * h:128 * (h + 1), :])
                    r = redp.tile([128, 1], FP, tag="r")
                    scr = redp.tile([128, FPAIR], BF, tag="scr", bufs=1)
                    nc.scalar.activation(scr[:], xb[:],
                                         mybir.ActivationFunctionType.Copy,
                                         accum_out=r[:])
                    nc.vector.tensor_add(s1[:, h:h + 1], s1[:, h:h + 1], r[:])

                    last_u = (p == NP_ - 1)
                    if u in DMA_T_UNITS:
                        xbT = xbtp.tile([128, NCH, 128], BF, tag="xbT")
                        nc.sync.dma_start_transpose(xbT[:], xb[:])
                        for j in range(NCH):
                            sl = xbT[:, j, :]
                            nc.tensor.matmul(
                                cov[h][:], sl, sl,
                                start=not started[h],
                                stop=last_u and j == NCH - 1,
                                skip_group_check=True)
                            started[h] = True
                    else:
                        for q in range(13):        # 49 = 12*4 + 1 chunks
                            nch = 4 if q < 12 else 1
                            pt = ptp.tile([128, nch * 128], BF, tag="pt")
                            for jj in range(nch):
                                m0 = 128 * (4 * q + jj)
                                nc.tensor.transpose(
                                    pt[:, 128 * jj:128 * (jj + 1)],
                                    xb[:, m0:m0 + 128], eye[:])
                            xtq = xtqp.tile([128, nch * 128], BF, tag="xtq")
                            nc.vector.tensor_copy(xtq[:], pt[:])
                            for jj in range(nch):
                                sl = xtq[:, 128 * jj:128 * (jj + 1)]
                                nc.tensor.matmul(
                                    cov[h][:], sl, sl,
                                    start=not started[h],
                                    stop=(last_u and q == 12 and jj == nch - 1),
                                    skip_group_check=True)
                                started[h] = True

            s2sb = accp.tile([128, 256], FP)
            for h in (0, 1):
                nc.vector.tensor_copy(s2sb[:, 128 * h:128 * (h + 1)], cov[h][:])
                nc.sync.dma_start(S2_d.ap()[h], s2sb[:, 128 * h:128 * (h + 1)])
            nc.sync.dma_start(S1_d.ap(), s1[:])

    nc.compile()
    return nc


def _build_pass_b():
    nc = bacc.Bacc("TRN2", target_bir_lowering=False, debug=False,
                   num_devices=N_CORES)
    X_d = nc.dram_tensor("X", [NL, C, HW], FP, kind="ExternalInput")
    wm_d = nc.dram_tensor("wm", [128, 256], FP, kind="ExternalInput")
    sc_d = nc.dram_tensor("sc", [128, 2], FP, kind="ExternalInput")
    sh_d = nc.dram_tensor("sh", [128, 2], FP, kind="ExternalInput")
    Xn_d = nc.dram_tensor("Xn", [NL, C, HW], FP, kind="ExternalOutput")
    X = X_d.ap()
    Xn = Xn_d.ap()

    KT = 448                   # matmul free-dim tile (14 * 448 = 6272)
    NK = FPAIR // KT

    with tile.TileContext(nc) as tc:
        with (
            tc.tile_pool(name="const", bufs=1) as constp,
            tc.tile_pool(name="xin", bufs=3) as xp,
            tc.tile_pool(name="xout", bufs=3) as op,
            tc.tile_pool(name="ps", bufs=4, space="PSUM") as psp,
        ):
            wm = constp.tile([128, 256], FP)
            nc.sync.dma_start(wm[:], wm_d.ap())
            sc = constp.tile([128, 2], FP)
            nc.sync.dma_start(sc[:], sc_d.ap())
            sh = constp.tile([128, 2], FP)
            nc.sync.dma_start(sh[:], sh_d.ap())

            for h in (0, 1):
                for p in range(NP_):
                    xf = xp.tile([128, FPAIR], FP, tag="x")
                    for i in (0, 1):
                        nc.sync.dma_start(
                            xf[:, HW * i:HW * (i + 1)],
                            X[2 * p + i, 128 * h:128 * (h + 1), :])
                    ot = op.tile([128, FPAIR], FP, tag="o")
                    for k in range(NK):
                        ps = psp.tile([128, KT], FP, tag="ps")
                        nc.tensor.matmul(
                            ps[:], wm[:, 128 * h:128 * (h + 1)],
                            xf[:, KT * k:KT * (k + 1)])
                        nc.scalar.activation(
                            ot[:, KT * k:KT * (k + 1)], ps[:],
                            mybir.ActivationFunctionType.Identity,
                            bias=sh[:, h:h + 1], scale=sc[:, h:h + 1])
                    for i in (0, 1):
                        nc.sync.dma_start(
                            Xn[2 * p + i, 128 * h:128 * (h + 1), :],
                            ot[:, HW * i:HW * (i + 1)])

    nc.compile()
    return nc


_PROGS = {}


def _programs():
    if "a" not in _PROGS:
        _PROGS["a"] = _build_pass_a()
        _PROGS["b"] = _build_pass_b()
    return _PROGS["a"], _PROGS["b"]


def kernel(X, weight, bias, _return_results=False):
    X = np.asarray(X, dtype=np.float32)
    weight = np.asarray(weight, dtype=np.float32).reshape(C)
    bias = np.asarray(bias, dtype=np.float32).reshape(C)
    nc_a, nc_b = _programs()

    Xr = X.reshape(N, C, HW)
    shards = [Xr[NL * i:NL * (i + 1)] for i in range(N_CORES)]
    shards_bf = [s.astype(ml_dtypes.bfloat16) for s in shards]
    eye = np.eye(128, dtype=ml_dtypes.bfloat16)
    core_ids = list(range(N_CORES))

    res_a = run_bass_kernel_spmd(
        nc_a, [{"X": s, "eye": eye} for s in shards_bf], core_ids)

    # host reduction of the tiny per-core stats (f64 for cleanliness)
    s1 = np.zeros((128, 2), np.float64)
    s2 = np.zeros((2, 128, 128), np.float64)
    for r in res_a.results:
        s1 += r["S1"].astype(np.float64)
        s2 += r["S2"].astype(np.float64)

    mean = np.concatenate([s1[:, 0], s1[:, 1]]) / M_TOT          # [256]
    wm_bd = np.zeros((2, 128, 128), np.float64)
    for g in range(G):
        h, o = divmod(g, 128 // CG)
        o *= CG
        mg = mean[CG * g:CG * (g + 1)]
        sg = (s2[h][o:o + CG, o:o + CG] / M_TOT - np.outer(mg, mg)
              + EPS * np.eye(CG))
        lam, u = np.linalg.eigh(sg)
        wm_bd[h][o:o + CG, o:o + CG] = (u / np.sqrt(lam)) @ u.T

    wm_full = np.zeros((C, C), np.float64)
    wm_full[:128, :128] = wm_bd[0]
    wm_full[128:, 128:] = wm_bd[1]
    v = wm_full @ mean                                           # [256]
    scale = weight.astype(np.float64)
    shift = bias.astype(np.float64) - scale * v

    wm_in = np.concatenate([wm_bd[0], wm_bd[1]], axis=1).astype(np.float32)
    sc_in = np.stack([scale[:128], scale[128:]], axis=1).astype(np.float32)
    sh_in = np.stack([shift[:128], shift[128:]], axis=1).astype(np.float32)

    res_b = run_bass_kernel_spmd(
        nc_b,
        [{"X": s, "wm": wm_in, "sc": sc_in, "sh": sh_in} for s in shards],
        core_ids)

    out = np.concatenate([r["Xn"] for r in res_b.results], axis=0)
    out = out.reshape(N, C, H, W).astype(np.float32)
    if _return_results:
        return out, (res_a, res_b)
    return out
